# revision 42
# baseline (speedup 1.0000x reference)
"""Trainium2 Bass kernel for NeuralDisCoCirc forward pass.

Problem: L=8 sequential layers; each layer, per sample b:
    z = h @ W[l,b] + bias[l,b];  h = where(mask[l,b], relu(z), z)
Shapes: x [16,1024] f32, weights [8,16,1024,1024] f32,
        biases/masks [8,16,1024].

Strategy (data-parallel over batch, 2 samples per core, 8 cores;
~105-110us vs the 213us fp32 baseline; rel err ~4.4e-3 vs 2e-2 gate):
  - Weights stream as bf16 (host cast): 32 MB per core instead of 64,
    halving the HBM traffic.  The kernel is stream-bound: ~76us of
    weight DMA at the observed ~420 GB/s per-core rate, plus startup
    and a short PE tail.
  - Host lays each core's weight shard as [t=l*2+b, p, jb*4096 +
    ki*512 + j'] with W row i = ki*128+p (chunk-major) and the columns
    split by output half (jb), so each matmul group depends on exactly
    one contiguous 1 MB half-tile DMA.  Both HWDGE rings stream halves
    concurrently; tiles 1..9 roll through a 4-deep wa/wb slot window
    (non-binding gates), tiles 10..15 sit in dedicated SBUF buffers so
    the tail of the stream has no slot-recycle gating; early tiles
    0..2 are quarter-split across both rings to shorten pipeline fill.
  - h lives in chunk-major column layout ([p, c], element i = c*128+p)
    and is the matmul stationary operand.  Per tile: 16 accumulating
    matmuls (h chunk [128,1] stationary, W chunk [128,512] bf16
    moving), the two PSUM banks' groups interleaved so both stops land
    adjacent and only one subsequent LDWEIGHTS pays the array drain.
  - The z row->column flip runs on the PE as 8 REGULAR matmuls
    (out[128,1] = zrow_chunk[1,128].T @ ones[1,1]) — transpose mode is
    avoided (each mode toggle costs a ~0.4us quiesce), and the SWDGE
    scatter it replaces cost 4-6us of software-DMA latency per tile
    (the original bottleneck: PE idled waiting for h).
  - All flips + bias/relu of tile t are deferred into tile t+1's MM
    stream; bias + masked relu (h = zb - mask*min(zb,0)) runs per jb
    half on [128,4] columns so the next tile's first LDWEIGHTS (which
    only needs h chunks 0-3) never waits on the full chain.
  - Final-layer tiles keep split MM groups (jb0's chain overlaps jb1's
    group) and ship the output per half, in column layout, over both
    rings; the host transposes it back.
"""

import numpy as np

import concourse.bass as bass
import concourse.mybir as mybir
from concourse import bacc
from concourse.tile import TileContext
from concourse.bass_utils import run_bass_kernel_spmd

L = 8          # layers
B = 16         # full batch
D = 1024       # width
NCORES = 8
BC = B // NCORES   # samples per core (2)
NT = L * BC        # (layer, sample) tiles per core (16)
KI = D // 128      # 8 chunks of 128 along the contraction dim
P = 128

F32 = mybir.dt.float32
BF16 = mybir.dt.bfloat16

WDT = BF16   # weight / h dtype on device
ZDT = BF16   # zrow dtype for the PE transposes (bf16: 1 cyc/row)

_CACHE = {}


def _build() -> bass.Bass:
    nc = bacc.Bacc("TRN2", target_bir_lowering=False, debug=False)
    w = nc.declare_dram_parameter("w", [NT, P, KI * D], WDT, isOutput=False)
    x = nc.declare_dram_parameter("x", [P, BC * KI], WDT, isOutput=False)
    bm = nc.declare_dram_parameter("bm", [P, NT * 2 * KI], F32, isOutput=False)
    # output ships in column layout, one contiguous [P, KH] block per
    # (sample, jb half); the host transposes it back to row layout.
    out = nc.declare_dram_parameter("out", [BC, 2, P, KI // 2], F32,
                                    isOutput=True)

    with TileContext(nc) as tc:
        with (
            tc.tile_pool(name="wp", bufs=4) as wp,  # per-tag: 4 x 1MB x 2 tags
            tc.tile_pool(name="const", bufs=1) as cp,
            tc.tile_pool(name="hrow", bufs=4) as hrp,
            tc.tile_pool(name="hcol", bufs=4) as hcp,
            tc.tile_pool(name="psr", bufs=2, space="PSUM") as psr,
            tc.tile_pool(name="psc", bufs=2, space="PSUM") as psc,
        ):
            # x first, split across both HWDGE rings: tiny (4KB) and it
            # gates the very first matmul.
            xt = cp.tile([P, BC * KI], WDT, tag="x")
            nc.sync.dma_start(out=xt[:, :KI], in_=x[:, :KI])
            nc.scalar.dma_start(out=xt[:, KI:], in_=x[:, KI:])

            KH = KI // 2  # ki chunks per half-tile
            LAST = NT - 1
            ST = 512      # starter block columns for tile 0 (fast first MM)
            wtiles = {}
            # Tiles 0..9 roll through a 4-deep wa/wb window (gates are
            # non-binding: slot t-4 is always consumed before tile t's
            # stream position comes up).  Tiles 10..15 get DEDICATED
            # buffers so the tail of the stream has no slot gating at
            # all: the full 32 MB streams back-to-back at line rate.
            NDED = 6
            for t in range(NT):
                if t < 3:
                    # early tiles: quarter-split each half across BOTH
                    # rings so the jb0 half (which gates the tile's first
                    # matmul group) arrives at full 2-ring rate instead
                    # of single-ring rate — shaves ~2us per early tile
                    # off the pipeline-fill phase.
                    wa = wp.tile([P, KH * D], WDT, tag="wa")
                    wb = wp.tile([P, KH * D], WDT, tag="wb")
                    HF = KH * D // 2
                    if t == 0:
                        nc.sync.dma_start(out=wa[:, :ST], in_=w[t, :, :ST])
                        nc.scalar.dma_start(
                            out=wa[:, HF:HF + ST],
                            in_=w[t, :, HF:HF + ST])
                        nc.sync.dma_start(
                            out=wa[:, ST:HF], in_=w[t, :, ST:HF])
                        nc.scalar.dma_start(
                            out=wa[:, HF + ST:], in_=w[t, :, HF + ST:KH * D])
                    else:
                        nc.sync.dma_start(out=wa[:, :HF], in_=w[t, :, :HF])
                        nc.scalar.dma_start(
                            out=wa[:, HF:], in_=w[t, :, HF:KH * D])
                    nc.sync.dma_start(
                        out=wb[:, :HF], in_=w[t, :, KH * D:KH * D + HF])
                    nc.scalar.dma_start(
                        out=wb[:, HF:], in_=w[t, :, KH * D + HF:])
                    wtiles[t] = (wa, wb)
                elif t >= NT - NDED:
                    wla = cp.tile([P, KH * D], WDT, tag=f"wd{t}a")
                    wlb = cp.tile([P, KH * D], WDT, tag=f"wd{t}b")
                    ea, eb = (nc.sync, nc.scalar) if t % 2 == 0 else (
                        nc.scalar, nc.sync)
                    ea.dma_start(out=wla, in_=w[t, :, : KH * D])
                    eb.dma_start(out=wlb, in_=w[t, :, KH * D:])
                    wtiles[t] = (wla, wlb)
                else:
                    # two 1MB half-tiles, one per HWDGE ring; alternate
                    # ring assignment per tile so slot-release skew
                    # doesn't pile up on one ring
                    wa = wp.tile([P, KH * D], WDT, tag="wa")
                    wb = wp.tile([P, KH * D], WDT, tag="wb")
                    ea, eb = (nc.sync, nc.scalar) if t % 2 == 0 else (
                        nc.scalar, nc.sync)
                    ea.dma_start(out=wa, in_=w[t, :, : KH * D])
                    eb.dma_start(out=wb, in_=w[t, :, KH * D:])
                    wtiles[t] = (wa, wb)

            bmt = cp.tile([P, NT * 2 * KI], F32, tag="bm")
            nc.gpsimd.dma_start(out=bmt, in_=bm[:])

            # [1,1] identity for PE row->column transposes
            idf = cp.tile([1, 1], ZDT, tag="idf")
            nc.vector.memset(idf, 1.0)

            h = [xt[:, b * KI:(b + 1) * KI] for b in range(BC)]

            def mm_group(t, prow, jb):
                # weight halves are jb-split: wtiles[t][jb][:, ki*512:...]
                cur = h[t % BC]
                wh = wtiles[t][jb]
                for ki in range(KI):
                    nc.tensor.matmul(
                        prow[0:1, jb * 512:(jb + 1) * 512],
                        lhsT=cur[:, ki:ki + 1],
                        rhs=wh[:, ki * 512:(ki + 1) * 512],
                        start=(ki == 0),
                        stop=(ki == KI - 1),
                    )

            def mm_interleaved(t, prow):
                # both accumulation groups interleaved (jb0/jb1 per ki):
                # the two group stops land adjacent at the end, so only
                # ONE subsequent LDWEIGHTS pays the array-drain penalty
                # per tile instead of two.
                cur = h[t % BC]
                for ki in range(KI):
                    for jb in range(2):
                        nc.tensor.matmul(
                            prow[0:1, jb * 512:(jb + 1) * 512],
                            lhsT=cur[:, ki:ki + 1],
                            rhs=wtiles[t][jb][:, ki * 512:(ki + 1) * 512],
                            start=(ki == 0),
                            stop=(ki == KI - 1),
                        )

            def transpose_half(zrow, pcol, jb):
                # Row->column flip as a REGULAR matmul (out[128,1] =
                # zrow_chunk[1,128].T @ ones[1,1]) so the PE never toggles
                # transpose mode (each toggle costs a ~0.4us array
                # quiesce).  pcol is [P, KI] f32 in PSUM.
                for c in range(4):
                    col = jb * 4 + c
                    nc.tensor.matmul(
                        pcol[:, col:col + 1],
                        lhsT=zrow[0:1, c * 128:(c + 1) * 128],
                        rhs=idf,
                        start=True,
                        stop=True,
                    )

            hnew_tiles = {}

            def finish_half(t, pcol, jb, final):
                # bias + masked relu in column layout, one jb half at a
                # time (the next tile's first LDWEIGHTS only needs h
                # chunks 0-3, so finishing per-half removes the h-chain
                # wait from the PE critical path):
                #   zb = z + bias;  h = zb - mask * min(zb, 0)
                sl = slice(jb * 4, (jb + 1) * 4)
                bias_ap = bmt[:, t * 2 * KI + jb * 4:
                              t * 2 * KI + (jb + 1) * 4]
                mask_ap = bmt[:, t * 2 * KI + KI + jb * 4:
                              t * 2 * KI + KI + (jb + 1) * 4]
                zb = hcp.tile([P, 4], F32, tag="zb")
                nc.vector.tensor_add(out=zb, in0=pcol[:, sl], in1=bias_ap)
                tmp = hcp.tile([P, 4], F32, tag="tmp")
                nc.vector.scalar_tensor_tensor(
                    out=tmp,
                    in0=zb,
                    scalar=0.0,
                    in1=mask_ap,
                    op0=mybir.AluOpType.min,
                    op1=mybir.AluOpType.mult,
                )
                if final:
                    b = t % BC
                    if jb == 0:
                        ho = hcp.tile([P, KI], F32, tag="ho")
                        hnew_tiles[t] = ho
                    ho = hnew_tiles[t]
                    nc.vector.tensor_sub(out=ho[:, sl], in0=zb, in1=tmp)
                    eng = nc.scalar if t == NT - 1 else nc.sync
                    eng.dma_start(out=out[b, jb], in_=ho[:, sl])
                else:
                    if jb == 0:
                        hnew = hcp.tile([P, KI], WDT, tag="h")
                        hnew_tiles[t] = hnew
                        h[t % BC] = hnew
                    hnew = hnew_tiles[t]
                    nc.vector.tensor_sub(out=hnew[:, sl], in0=zb, in1=tmp)

            # pending = (t, zrow0, zrow1, pcol): ALL flips + bias/relu of
            # a non-final tile, deferred into the next tile's MM stream.
            pending = None

            def flush_pending():
                nonlocal pending
                if pending is None:
                    return
                pt, pz0, pz1, ppc = pending
                transpose_half(pz0, ppc, 0)
                finish_half(pt, ppc, 0, final=False)
                transpose_half(pz1, ppc, 1)
                finish_half(pt, ppc, 1, final=False)
                pending = None

            for t in range(NT):
                final = (t >= NT - BC)   # last layer tiles
                prow = psr.tile([1, D], F32)

                if not final:
                    mm_interleaved(t, prow)
                    flush_pending()
                    zrow0 = hrp.tile([1, 512], ZDT, tag="zr0")
                    nc.vector.tensor_copy(out=zrow0, in_=prow[0:1, 0:512])
                    zrow1 = hrp.tile([1, 512], ZDT, tag="zr1")
                    nc.vector.tensor_copy(out=zrow1, in_=prow[0:1, 512:1024])
                    pcol = psc.tile([P, KI], F32)
                    pending = (t, zrow0, zrow1, pcol)
                else:
                    # final tiles keep split groups: the jb0 CAST/flips
                    # overlap the jb1 group, shortening the exposed tail.
                    mm_group(t, prow, 0)
                    flush_pending()
                    zrow0 = hrp.tile([1, 512], ZDT, tag="zr0")
                    nc.vector.tensor_copy(out=zrow0, in_=prow[0:1, 0:512])
                    mm_group(t, prow, 1)
                    pcol = psc.tile([P, KI], F32)
                    transpose_half(zrow0, pcol, 0)
                    zrow1 = hrp.tile([1, 512], ZDT, tag="zr1")
                    nc.vector.tensor_copy(out=zrow1, in_=prow[0:1, 512:1024])
                    finish_half(t, pcol, 0, final=True)
                    transpose_half(zrow1, pcol, 1)
                    finish_half(t, pcol, 1, final=True)

            assert pending is None
    nc.finalize()
    return nc


def _get_nc():
    if "nc" not in _CACHE:
        _CACHE["nc"] = _build()
    return _CACHE["nc"]


def _prep_core_inputs(c, x, weights, biases, masks):
    import ml_dtypes
    b0 = c * BC
    # weights[l, b, i, j], i = ki*128 + p  ->  [t, p, jb*4096 + ki*512 + j']
    # (jb-split halves: each matmul group depends on only one 1MB half)
    wc = weights[:, b0:b0 + BC].reshape(NT, KI, P, 2, 512)
    wc = np.ascontiguousarray(wc.transpose(0, 2, 3, 1, 4)).reshape(
        NT, P, KI * D)
    wc = wc.astype(ml_dtypes.bfloat16)
    # x[b, c*128+p] -> [p, b*KI + c]
    xc = x[b0:b0 + BC].reshape(BC, KI, P)
    xc = np.ascontiguousarray(xc.transpose(2, 0, 1)).reshape(P, BC * KI)
    xc = xc.astype(ml_dtypes.bfloat16)
    # bias/mask [l, b, c*128+p] -> [p, (t, {bias,mask}, c)]
    bc = biases[:, b0:b0 + BC].reshape(L, BC, KI, P).transpose(3, 0, 1, 2)
    mc = masks[:, b0:b0 + BC].astype(np.float32).reshape(L, BC, KI, P)
    mc = mc.transpose(3, 0, 1, 2)
    bmc = np.stack([bc, mc], axis=3)  # [p, L, BC, 2, KI]
    bmc = np.ascontiguousarray(bmc).reshape(P, NT * 2 * KI)
    return {"w": wc, "x": xc, "bm": bmc}


def _run(inputs: dict, trace: bool = False, trace_cores=None):
    x = np.asarray(inputs["x"], dtype=np.float32)
    weights = np.asarray(inputs["weights"], dtype=np.float32)
    biases = np.asarray(inputs["biases"], dtype=np.float32)
    masks = np.asarray(inputs["masks"])

    nc = _get_nc()
    in_maps = [
        _prep_core_inputs(c, x, weights, biases, masks) for c in range(NCORES)
    ]
    kw = {}
    if trace_cores is not None:
        kw["trace_cores"] = trace_cores
    res = run_bass_kernel_spmd(
        nc, in_maps, core_ids=list(range(NCORES)), trace=trace, **kw
    )
    outs = []
    for c in range(NCORES):
        oc = res.results[c]["out"]  # [BC, 2, P, KH] column layout
        # full[b, (jb*KH + k)*128 + p] = oc[b, jb, p, k]
        oc = oc.transpose(0, 1, 3, 2).reshape(BC, D)
        outs.append(oc)
    full = np.concatenate(outs, axis=0).astype(np.float32)
    return full, res


def kernel(**inputs) -> np.ndarray:
    full, _ = _run(inputs, trace=False)
    return full


# revision 45
# speedup vs baseline: 1.0455x; 1.0455x over previous
"""Trainium2 Bass kernel for NeuralDisCoCirc forward pass.

Problem: L=8 sequential layers; each layer, per sample b:
    z = h @ W[l,b] + bias[l,b];  h = where(mask[l,b], relu(z), z)
Shapes: x [16,1024] f32, weights [8,16,1024,1024] f32,
        biases/masks [8,16,1024].

Strategy (data-parallel over batch, 2 samples per core, 8 cores;
~105-110us vs the 213us fp32 baseline; rel err ~4.4e-3 vs 2e-2 gate):
  - Weights stream as bf16 (host cast): 32 MB per core instead of 64,
    halving the HBM traffic.  The kernel is stream-bound: ~76us of
    weight DMA at the observed ~420 GB/s per-core rate, plus startup
    and a short PE tail.
  - Host lays each core's weight shard as [t=l*2+b, p, jb*4096 +
    ki*512 + j'] with W row i = ki*128+p (chunk-major) and the columns
    split by output half (jb), so each matmul group depends on exactly
    one contiguous 1 MB half-tile DMA.  Both HWDGE rings stream halves
    concurrently; tiles 1..9 roll through a 4-deep wa/wb slot window
    (non-binding gates), tiles 10..15 sit in dedicated SBUF buffers so
    the tail of the stream has no slot-recycle gating; early tiles
    0..2 are quarter-split across both rings to shorten pipeline fill.
  - h lives in chunk-major column layout ([p, c], element i = c*128+p)
    and is the matmul stationary operand.  Per tile: 16 accumulating
    matmuls (h chunk [128,1] stationary, W chunk [128,512] bf16
    moving), the two PSUM banks' groups interleaved so both stops land
    adjacent and only one subsequent LDWEIGHTS pays the array drain.
  - The z row->column flip runs on the PE as 8 REGULAR matmuls
    (out[128,1] = zrow_chunk[1,128].T @ ones[1,1]) — transpose mode is
    avoided (each mode toggle costs a ~0.4us quiesce), and the SWDGE
    scatter it replaces cost 4-6us of software-DMA latency per tile
    (the original bottleneck: PE idled waiting for h).
  - All flips + bias/relu of tile t are deferred into tile t+1's MM
    stream; bias + masked relu (h = zb - mask*min(zb,0)) runs per jb
    half on [128,4] columns so the next tile's first LDWEIGHTS (which
    only needs h chunks 0-3) never waits on the full chain.
  - Final-layer tiles keep split MM groups (jb0's chain overlaps jb1's
    group) and ship the output per half, in column layout, over both
    rings; the host transposes it back.
"""

import numpy as np

import concourse.bass as bass
import concourse.mybir as mybir
from concourse import bacc
from concourse.tile import TileContext
from concourse.bass_utils import run_bass_kernel_spmd

L = 8          # layers
B = 16         # full batch
D = 1024       # width
NCORES = 8
BC = B // NCORES   # samples per core (2)
NT = L * BC        # (layer, sample) tiles per core (16)
KI = D // 128      # 8 chunks of 128 along the contraction dim
P = 128

F32 = mybir.dt.float32
BF16 = mybir.dt.bfloat16

WDT = BF16   # weight / h dtype on device
ZDT = BF16   # zrow dtype for the PE transposes (bf16: 1 cyc/row)

_CACHE = {}


def _build() -> bass.Bass:
    nc = bacc.Bacc("TRN2", target_bir_lowering=False, debug=False)
    w = nc.declare_dram_parameter("w", [NT, P, KI * D], WDT, isOutput=False)
    x = nc.declare_dram_parameter("x", [P, BC * KI], WDT, isOutput=False)
    bm = nc.declare_dram_parameter("bm", [P, NT * 2 * KI], F32, isOutput=False)
    # output ships in column layout, one contiguous [P, KH] block per
    # (sample, jb half); the host transposes it back to row layout.
    out = nc.declare_dram_parameter("out", [BC, 2, P, KI // 2], F32,
                                    isOutput=True)

    with TileContext(nc) as tc:
        with (
            tc.tile_pool(name="wp", bufs=4) as wp,  # per-tag: 4 x 1MB x 2 tags
            tc.tile_pool(name="const", bufs=1) as cp,
            tc.tile_pool(name="hrow", bufs=4) as hrp,
            tc.tile_pool(name="hcol", bufs=4) as hcp,
            tc.tile_pool(name="psr", bufs=2, space="PSUM") as psr,
            tc.tile_pool(name="psc", bufs=2, space="PSUM") as psc,
        ):
            # x first, split across both HWDGE rings: tiny (4KB) and it
            # gates the very first matmul.
            xt = cp.tile([P, BC * KI], WDT, tag="x")
            nc.sync.dma_start(out=xt[:, :KI], in_=x[:, :KI])
            nc.scalar.dma_start(out=xt[:, KI:], in_=x[:, KI:])

            KH = KI // 2  # ki chunks per half-tile
            LAST = NT - 1
            ST = 512      # starter block columns for tile 0 (fast first MM)
            wtiles = {}
            # Tiles 0..9 roll through a 4-deep wa/wb window (gates are
            # non-binding: slot t-4 is always consumed before tile t's
            # stream position comes up).  Tiles 10..15 get DEDICATED
            # buffers so the tail of the stream has no slot gating at
            # all: the full 32 MB streams back-to-back at line rate.
            NDED = 6
            for t in range(NT):
                if t < 3:
                    # early tiles: quarter-split each half across BOTH
                    # rings so the jb0 half (which gates the tile's first
                    # matmul group) arrives at full 2-ring rate instead
                    # of single-ring rate — shaves ~2us per early tile
                    # off the pipeline-fill phase.
                    wa = wp.tile([P, KH * D], WDT, tag="wa")
                    wb = wp.tile([P, KH * D], WDT, tag="wb")
                    HF = KH * D // 2
                    if t == 0:
                        nc.sync.dma_start(out=wa[:, :ST], in_=w[t, :, :ST])
                        nc.scalar.dma_start(
                            out=wa[:, HF:HF + ST],
                            in_=w[t, :, HF:HF + ST])
                        nc.sync.dma_start(
                            out=wa[:, ST:HF], in_=w[t, :, ST:HF])
                        nc.scalar.dma_start(
                            out=wa[:, HF + ST:], in_=w[t, :, HF + ST:KH * D])
                    else:
                        nc.sync.dma_start(out=wa[:, :HF], in_=w[t, :, :HF])
                        nc.scalar.dma_start(
                            out=wa[:, HF:], in_=w[t, :, HF:KH * D])
                    nc.sync.dma_start(
                        out=wb[:, :HF], in_=w[t, :, KH * D:KH * D + HF])
                    nc.scalar.dma_start(
                        out=wb[:, HF:], in_=w[t, :, KH * D + HF:])
                    wtiles[t] = (wa, wb)
                elif t >= NT - NDED:
                    wla = cp.tile([P, KH * D], WDT, tag=f"wd{t}a")
                    wlb = cp.tile([P, KH * D], WDT, tag=f"wd{t}b")
                    ea, eb = (nc.sync, nc.scalar) if t % 2 == 0 else (
                        nc.scalar, nc.sync)
                    ea.dma_start(out=wla, in_=w[t, :, : KH * D])
                    eb.dma_start(out=wlb, in_=w[t, :, KH * D:])
                    wtiles[t] = (wla, wlb)
                else:
                    # two 1MB half-tiles, one per HWDGE ring; alternate
                    # ring assignment per tile so slot-release skew
                    # doesn't pile up on one ring
                    wa = wp.tile([P, KH * D], WDT, tag="wa")
                    wb = wp.tile([P, KH * D], WDT, tag="wb")
                    ea, eb = (nc.sync, nc.scalar) if t % 2 == 0 else (
                        nc.scalar, nc.sync)
                    ea.dma_start(out=wa, in_=w[t, :, : KH * D])
                    eb.dma_start(out=wb, in_=w[t, :, KH * D:])
                    wtiles[t] = (wa, wb)

            bmt = cp.tile([P, NT * 2 * KI], F32, tag="bm")
            nc.gpsimd.dma_start(out=bmt, in_=bm[:])

            # [1,1] identity for PE row->column transposes
            idf = cp.tile([1, 1], ZDT, tag="idf")
            nc.vector.memset(idf, 1.0)

            h = [xt[:, b * KI:(b + 1) * KI] for b in range(BC)]

            def mm_group(t, prow, jb):
                # weight halves are jb-split: wtiles[t][jb][:, ki*512:...]
                cur = h[t % BC]
                wh = wtiles[t][jb]
                for ki in range(KI):
                    nc.tensor.matmul(
                        prow[0:1, jb * 512:(jb + 1) * 512],
                        lhsT=cur[:, ki:ki + 1],
                        rhs=wh[:, ki * 512:(ki + 1) * 512],
                        start=(ki == 0),
                        stop=(ki == KI - 1),
                    )

            def mm_interleaved(t, prow):
                # both accumulation groups interleaved (jb0/jb1 per ki):
                # the two group stops land adjacent at the end, so only
                # ONE subsequent LDWEIGHTS pays the array-drain penalty
                # per tile instead of two.
                cur = h[t % BC]
                for ki in range(KI):
                    for jb in range(2):
                        nc.tensor.matmul(
                            prow[0:1, jb * 512:(jb + 1) * 512],
                            lhsT=cur[:, ki:ki + 1],
                            rhs=wtiles[t][jb][:, ki * 512:(ki + 1) * 512],
                            start=(ki == 0),
                            stop=(ki == KI - 1),
                        )

            def transpose_half(zrow, pcol, jb):
                # Row->column flip as a REGULAR matmul (out[128,1] =
                # zrow_chunk[1,128].T @ ones[1,1]) so the PE never toggles
                # transpose mode (each toggle costs a ~0.4us array
                # quiesce).  pcol is [P, KI] f32 in PSUM.
                for c in range(4):
                    col = jb * 4 + c
                    nc.tensor.matmul(
                        pcol[:, col:col + 1],
                        lhsT=zrow[0:1, c * 128:(c + 1) * 128],
                        rhs=idf,
                        start=True,
                        stop=True,
                    )

            hnew_tiles = {}

            def finish_half(t, pcol, jb, final):
                # bias + masked relu in column layout, one jb half at a
                # time (the next tile's first LDWEIGHTS only needs h
                # chunks 0-3, so finishing per-half removes the h-chain
                # wait from the PE critical path):
                #   zb = z + bias;  h = zb - mask * min(zb, 0)
                sl = slice(jb * 4, (jb + 1) * 4)
                bias_ap = bmt[:, t * 2 * KI + jb * 4:
                              t * 2 * KI + (jb + 1) * 4]
                mask_ap = bmt[:, t * 2 * KI + KI + jb * 4:
                              t * 2 * KI + KI + (jb + 1) * 4]
                zb = hcp.tile([P, 4], F32, tag="zb")
                nc.vector.tensor_add(out=zb, in0=pcol[:, sl], in1=bias_ap)
                tmp = hcp.tile([P, 4], F32, tag="tmp")
                nc.vector.scalar_tensor_tensor(
                    out=tmp,
                    in0=zb,
                    scalar=0.0,
                    in1=mask_ap,
                    op0=mybir.AluOpType.min,
                    op1=mybir.AluOpType.mult,
                )
                if final:
                    b = t % BC
                    if jb == 0:
                        ho = hcp.tile([P, KI], F32, tag="ho")
                        hnew_tiles[t] = ho
                    ho = hnew_tiles[t]
                    nc.vector.tensor_sub(out=ho[:, sl], in0=zb, in1=tmp)
                    eng = nc.scalar if t == NT - 1 else nc.sync
                    eng.dma_start(out=out[b, jb], in_=ho[:, sl])
                else:
                    if jb == 0:
                        hnew = hcp.tile([P, KI], WDT, tag="h")
                        hnew_tiles[t] = hnew
                        h[t % BC] = hnew
                    hnew = hnew_tiles[t]
                    nc.vector.tensor_sub(out=hnew[:, sl], in0=zb, in1=tmp)

            # pending = (t, zrow0, zrow1, pcol): ALL flips + bias/relu of
            # a non-final tile, deferred into the next tile's MM stream.
            pending = None

            def flush_pending():
                nonlocal pending
                if pending is None:
                    return
                pt, pz0, pz1, ppc = pending
                transpose_half(pz0, ppc, 0)
                finish_half(pt, ppc, 0, final=False)
                transpose_half(pz1, ppc, 1)
                finish_half(pt, ppc, 1, final=False)
                pending = None

            for t in range(NT):
                final = (t >= NT - BC)   # last layer tiles
                prow = psr.tile([1, D], F32)

                if not final:
                    mm_interleaved(t, prow)
                    flush_pending()
                    zrow0 = hrp.tile([1, 512], ZDT, tag="zr0")
                    nc.vector.tensor_copy(out=zrow0, in_=prow[0:1, 0:512])
                    zrow1 = hrp.tile([1, 512], ZDT, tag="zr1")
                    nc.vector.tensor_copy(out=zrow1, in_=prow[0:1, 512:1024])
                    pcol = psc.tile([P, KI], F32)
                    pending = (t, zrow0, zrow1, pcol)
                else:
                    # final tiles keep split groups: the jb0 CAST/flips
                    # overlap the jb1 group, shortening the exposed tail.
                    mm_group(t, prow, 0)
                    flush_pending()
                    zrow0 = hrp.tile([1, 512], ZDT, tag="zr0")
                    nc.vector.tensor_copy(out=zrow0, in_=prow[0:1, 0:512])
                    mm_group(t, prow, 1)
                    pcol = psc.tile([P, KI], F32)
                    transpose_half(zrow0, pcol, 0)
                    zrow1 = hrp.tile([1, 512], ZDT, tag="zr1")
                    nc.vector.tensor_copy(out=zrow1, in_=prow[0:1, 512:1024])
                    finish_half(t, pcol, 0, final=True)
                    transpose_half(zrow1, pcol, 1)
                    finish_half(t, pcol, 1, final=True)

            assert pending is None
    nc.finalize()
    return nc


def _get_nc():
    if "nc" not in _CACHE:
        _CACHE["nc"] = _build()
    return _CACHE["nc"]


def _prep_core_inputs(c, x, weights, biases, masks):
    import ml_dtypes
    b0 = c * BC
    # weights[l, b, i, j], i = ki*128 + p  ->  [t, p, jb*4096 + ki*512 + j']
    # (jb-split halves: each matmul group depends on only one 1MB half)
    wc = weights[:, b0:b0 + BC].reshape(NT, KI, P, 2, 512)
    wc = np.ascontiguousarray(wc.transpose(0, 2, 3, 1, 4)).reshape(
        NT, P, KI * D)
    wc = wc.astype(ml_dtypes.bfloat16)
    # x[b, c*128+p] -> [p, b*KI + c]
    xc = x[b0:b0 + BC].reshape(BC, KI, P)
    xc = np.ascontiguousarray(xc.transpose(2, 0, 1)).reshape(P, BC * KI)
    xc = xc.astype(ml_dtypes.bfloat16)
    # bias/mask [l, b, c*128+p] -> [p, (t, {bias,mask}, c)]
    bc = biases[:, b0:b0 + BC].reshape(L, BC, KI, P).transpose(3, 0, 1, 2)
    mc = masks[:, b0:b0 + BC].astype(np.float32).reshape(L, BC, KI, P)
    mc = mc.transpose(3, 0, 1, 2)
    bmc = np.stack([bc, mc], axis=3)  # [p, L, BC, 2, KI]
    bmc = np.ascontiguousarray(bmc).reshape(P, NT * 2 * KI)
    return {"w": wc, "x": xc, "bm": bmc}


def _run(inputs: dict, trace: bool = False, trace_cores=None):
    x = np.asarray(inputs["x"], dtype=np.float32)
    weights = np.asarray(inputs["weights"], dtype=np.float32)
    biases = np.asarray(inputs["biases"], dtype=np.float32)
    masks = np.asarray(inputs["masks"])

    nc = _get_nc()
    in_maps = [
        _prep_core_inputs(c, x, weights, biases, masks) for c in range(NCORES)
    ]
    kw = {}
    if trace_cores is not None:
        kw["trace_cores"] = trace_cores
    res = run_bass_kernel_spmd(
        nc, in_maps, core_ids=list(range(NCORES)), trace=trace, **kw
    )
    outs = []
    for c in range(NCORES):
        oc = res.results[c]["out"]  # [BC, 2, P, KH] column layout
        # full[b, (jb*KH + k)*128 + p] = oc[b, jb, p, k]
        oc = oc.transpose(0, 1, 3, 2).reshape(BC, D)
        outs.append(oc)
    full = np.concatenate(outs, axis=0).astype(np.float32)
    return full, res


def kernel(**inputs) -> np.ndarray:
    full, _ = _run(inputs, trace=False)
    return full
